# revision 13
# baseline (speedup 1.0000x reference)
"""Trainium2 Bass kernel for AccentVarianceAdaptor.

Computation (per batch row):
  pbin = searchsorted(linspace(50,400,256), clip(pitch,50,400), 'left')
  ebin = searchsorted(linspace(0,1,256),  clip(energy,0,1),  'left')
  y    = encoder + ptab[pbin] + etab[ebin]               # [S, H]
  dur  = max(round(duration), 1); cum = cumsum(dur)
  out[t] = y[searchsorted(cum, t, 'right')] * (t < cum[-1])  # [T, H]

Mapping to the hardware (one NeuronCore handles 4 batch rows):
  - table lookup: C[bin, tok] = (boundary[bin] < v[tok]) built with a K=1
    broadcast matmul + tensor_scalar(is_gt); then
    y = base + sum_half C_half.T @ dTab_half + encoder, where
    dTab[i] = tab[i+1] - tab[i] in bf16 (telescoping sum == row select).
  - durations: (d + 2^23) - 2^23 rounds half-to-even exactly in f32;
    cumulative sums via triangular matmuls with a PE-transpose supplying the
    inter-chunk offsets.
  - length-regulate: scatter 1.0 at delta[cum[j]] (indirect DMA); frame_idx =
    inclusive-prefix-sum(delta) via triangular matmuls in a 16-partition
    layout matching dma_gather's index format; dma_gather pulls bf16 y rows
    from HBM scratch (row 512 = zeros covers the ragged tail) using
    prepare_only descriptors spread over 4 SWDGE queues so transfers overlap;
    the scalar engine upcasts bf16->f32 and plain DMA stores the frames.
"""

import os
import sys

for _p in ("/opt/trn_rl_repo", "/root/.axon_site/_ro/trn_rl_repo"):
    if os.path.isdir(_p) and _p not in sys.path:
        sys.path.insert(0, _p)

import numpy as np

from concourse import bacc, mybir, tile
from concourse.bass import AP, IndirectOffsetOnAxis, ts
from concourse.bass_utils import run_bass_kernel_spmd

B, S, H = 32, 512, 256
NBINS = 256
T = 4096
NCORES = 8
BPC = B // NCORES  # batches per core
P = 128
NCH = S // P  # token chunks per batch
YROWS = S + 8  # y scratch rows per batch (512 tokens + zero rows)
DELTA_N = T + 8
GCHUNK = 1024  # max indices per dma_gather (SWDGE ring limit)
NGATHER = T // GCHUNK
NQ = 4  # SWDGE queues
F32 = mybir.dt.float32
BF16 = mybir.dt.bfloat16
I32 = mybir.dt.int32
I16 = mybir.dt.int16
A = mybir.AluOpType
ACT_COPY = mybir.ActivationFunctionType.Copy


def _boundaries():
    """Bit-exact copies of the f32 boundaries the jax reference uses."""
    import jax

    with jax.default_device(jax.devices("cpu")[0]):
        import jax.numpy as jnp

        bp = np.asarray(jnp.linspace(50.0, 400.0, NBINS), np.float32)
        be = np.asarray(jnp.linspace(0.0, 1.0, NBINS), np.float32)
    return bp, be


def _host_constants(pitch_table, energy_table):
    bp, be = _boundaries()
    consts = {}
    import ml_dtypes
    for name, tab in (("dpt", pitch_table), ("det", energy_table)):
        d = np.zeros((NBINS, H), np.float32)
        d[:-1] = tab[1:] - tab[:-1]  # f32 arithmetic, row 255 stays 0
        consts[name] = d.astype(ml_dtypes.bfloat16)
    consts["base"] = (pitch_table[0] + energy_table[0]).reshape(1, H)
    consts["bndp"] = bp.reshape(2, P).T.copy()  # [128, 2], col h = b[h*128 + p]
    consts["bnde"] = be.reshape(2, P).T.copy()
    j = np.arange(P, dtype=np.float32)
    consts["lt128"] = (j[:, None] <= j[None, :]).astype(np.float32)  # incl prefix
    consts["slt128"] = (j[:, None] < j[None, :]).astype(np.float32)  # excl prefix
    c4 = np.arange(NCH, dtype=np.float32)
    consts["slt4"] = (c4[:, None] < c4[None, :]).astype(np.float32)
    j16 = np.arange(16, dtype=np.float32)
    consts["lt16"] = (j16[:, None] <= j16[None, :]).astype(np.float32)
    consts["ones1"] = np.ones((1, P), np.float32)
    consts["ones1_16"] = np.ones((1, 16), np.float32)
    consts["onescol16"] = np.ones((16, 1), np.float32)
    consts["onecol"] = np.ones((P, 1), np.float32)
    consts["ones4"] = np.ones((P, NCH), np.float32)
    consts["ident"] = np.eye(P, dtype=np.float32)
    m = np.arange(P)
    consts["rep16"] = (m[None, :] % 16 == np.arange(16)[:, None]).astype(np.float32)
    consts["goff"] = (np.arange(T // GCHUNK, dtype=np.float32) * GCHUNK).reshape(1, -1)
    thr = np.full((P, T // 16), 511.5, np.float32)
    for g in range(T // GCHUNK):
        thr[0::16, g * (GCHUNK // 16)] = 1e9  # frame g*GCHUNK stays non-negative
    consts["thr"] = thr
    return consts


def build_nc():
    nc = bacc.Bacc(
        "TRN2",
        target_bir_lowering=False,
        debug=False,
        enable_asserts=False,
        num_swdge_queues=NQ,
    )

    enc_dr = nc.dram_tensor("enc", [BPC, S, H], F32, kind="ExternalInput")
    pit_dr = nc.dram_tensor("pitch", [BPC, S], F32, kind="ExternalInput")
    ene_dr = nc.dram_tensor("energy", [BPC, S], F32, kind="ExternalInput")
    dur_dr = nc.dram_tensor("durt", [BPC, S], F32, kind="ExternalInput")
    tab_dr = {
        nm: nc.dram_tensor(nm, [NBINS, H], BF16, kind="ExternalInput")
        for nm in ("dpt", "det")
    }
    cdr = {
        name: nc.dram_tensor(name, list(arr_shape), F32, kind="ExternalInput")
        for name, arr_shape in (
            ("base", (1, H)),
            ("bndp", (P, 2)),
            ("bnde", (P, 2)),
            ("lt128", (P, P)),
            ("slt128", (P, P)),
            ("slt4", (NCH, NCH)),
            ("lt16", (16, 16)),
            ("ones1", (1, P)),
            ("ones1_16", (1, 16)),
            ("onescol16", (16, 1)),
            ("onecol", (P, 1)),
            ("ones4", (P, NCH)),
            ("ident", (P, P)),
            ("rep16", (16, P)),
            ("goff", (1, NGATHER)),
            ("thr", (P, T // 16)),
        )
    }
    out_dr = [
        nc.dram_tensor(f"out{b}", [T, H], F32, kind="ExternalOutput")
        for b in range(BPC)
    ]
    y_dr = nc.dram_tensor("y_scr", [BPC * YROWS, H], BF16)
    delta_dr = [nc.dram_tensor(f"delta{b}", [DELTA_N, 1], F32) for b in range(BPC)]

    with tile.TileContext(nc) as tc:
        with (
            tc.tile_pool(name="const", bufs=1) as cp,
            tc.tile_pool(name="work", bufs=4) as wp,
            tc.tile_pool(name="gat", bufs=4) as gp,
            tc.tile_pool(name="gf32", bufs=4) as op,
            tc.tile_pool(name="idxp", bufs=BPC) as ip,
            tc.tile_pool(name="pbig", bufs=1, space="PSUM") as pb,
            tc.tile_pool(name="peps", bufs=2, space="PSUM") as pe,
            tc.tile_pool(name="psmall", bufs=2, space="PSUM") as psm,
            tc.tile_pool(name="pmicro", bufs=1, space="PSUM") as pmi,
            tc.tile_pool(name="prep", bufs=2, space="PSUM") as prp,
        ):
            # ---- constants ----
            csb = {}
            for name, dr in cdr.items():
                t_ = cp.tile(list(dr.shape), F32, tag=name)
                nc.sync.dma_start(out=t_[:], in_=dr[:])
                csb[name] = t_
            for nm, dr in tab_dr.items():
                t_ = cp.tile([P, 2, H], BF16, tag=nm)
                nc.sync.dma_start(
                    out=t_[:], in_=dr[:].rearrange("(h p) f -> p h f", p=P)
                )
                csb[nm] = t_
            zt = cp.tile([8, H], BF16)
            nc.gpsimd.memset(zt[:], 0.0)
            zrow = cp.tile([1, DELTA_N], F32)
            nc.gpsimd.memset(zrow[:], 0.0)

            idx_tiles = {}
            cnt_tiles = {}

            def phase0(b):
                # ---- dur load + frame-delta chain (tiny, unblocks gathers) ----
                dur_raw = wp.tile([P, NCH], F32, tag="draw")
                nc.sync.dma_start(
                    out=dur_raw[:], in_=dur_dr[b].rearrange("(c p) -> p c", p=P)
                )

                # ---- dur = max(round_half_even(durt), 1) ----
                MAGIC = float(1 << 23)
                dr0 = wp.tile([P, NCH], F32, tag="dr0")
                nc.vector.tensor_scalar(out=dr0[:], in0=dur_raw[:], scalar1=MAGIC, scalar2=MAGIC, op0=A.add, op1=A.subtract)
                dur_sb = wp.tile([P, NCH], F32, tag="dur")
                nc.vector.tensor_scalar(out=dur_sb[:], in0=dr0[:], scalar1=1.0, scalar2=None, op0=A.max)

                # ---- inclusive cum over tokens (wrapped j = pc*128 + p) ----
                i1_ps = psm.tile([P, NCH], F32, tag="small")
                nc.tensor.matmul(out=i1_ps[:], lhsT=csb["lt128"][:], rhs=dur_sb[:], start=True, stop=True)
                i1_sb = wp.tile([P, NCH], F32, tag="i1")
                nc.vector.tensor_copy(out=i1_sb[:], in_=i1_ps[:])
                tot_ps = psm.tile([NCH, P], F32, tag="small")
                nc.tensor.transpose(out=tot_ps[:], in_=i1_sb[:], identity=csb["ident"][:])
                tot_sb = wp.tile([NCH, P], F32, tag="tot")
                nc.vector.tensor_copy(out=tot_sb[:], in_=tot_ps[:])
                totb_sb = wp.tile([NCH, P], F32, tag="totb")
                nc.vector.tensor_copy(out=totb_sb[:], in_=tot_sb[:, P - 1 : P].to_broadcast([NCH, P]))
                cum_ps = psm.tile([P, NCH], F32, tag="small")
                nc.tensor.matmul(out=cum_ps[:], lhsT=csb["lt128"][:], rhs=dur_sb[:], start=True, stop=False)
                nc.tensor.matmul(out=cum_ps[:], lhsT=totb_sb[:], rhs=csb["slt4"][:], start=False, stop=True)
                cum_i32 = wp.tile([P, NCH], I32, tag="cumi")
                nc.vector.tensor_copy(out=cum_i32[:], in_=cum_ps[:])
                tot_row_ps = pmi.tile([1, NCH], F32, tag="micro")
                nc.tensor.matmul(out=tot_row_ps[:], lhsT=csb["onecol"][:], rhs=dur_sb[:], start=True, stop=True)
                tot1 = wp.tile([1, 1], F32, tag="tot1")
                nc.vector.tensor_reduce(out=tot1[:], in_=tot_row_ps[:], axis=mybir.AxisListType.X, op=A.add)
                cnt_f = wp.tile([1, NGATHER], F32, tag="cntf")
                nc.vector.tensor_tensor(
                    out=cnt_f[:],
                    in0=tot1[:].to_broadcast([1, NGATHER]),
                    in1=csb["goff"][:],
                    op=A.subtract,
                )
                cnt_i = wp.tile([1, NGATHER], I32, tag="cnti")
                nc.vector.tensor_scalar(out=cnt_i[:], in0=cnt_f[:], scalar1=1.0, scalar2=float(GCHUNK), op0=A.max, op1=A.min)
                cnt_tiles[b] = cnt_i

                # ---- delta: zero then scatter 1.0 at cum positions ----
                nc.sync.dma_start(
                    out=delta_dr[b][:].rearrange("v o -> o v"), in_=zrow[:]
                )
                for c in range(NCH):
                    nc.gpsimd.indirect_dma_start(
                        out=delta_dr[b][:],
                        out_offset=IndirectOffsetOnAxis(ap=cum_i32[:, c : c + 1], axis=0),
                        in_=csb["onecol"][:],
                        in_offset=None,
                    )

            def phase1(b):
                # ---- heavy loads + variance-adder compute ----
                enc_sb = wp.tile([P, NCH, H], F32, tag="enc")
                nc.sync.dma_start(
                    out=enc_sb[:], in_=enc_dr[b].rearrange("(c p) f -> p c f", p=P)
                )
                vp_row = wp.tile([1, S], F32, tag="vp")
                ve_row = wp.tile([1, S], F32, tag="ve")
                nc.sync.dma_start(out=vp_row[:], in_=pit_dr[b][None, :])
                nc.sync.dma_start(out=ve_row[:], in_=ene_dr[b][None, :])

                # ---- C matrices: C[bin_p, tok] = (boundary[bin] < v[tok]) ----
                vp_ps = pb.tile([P, S], F32, tag="vps")
                nc.tensor.matmul(out=vp_ps[:], lhsT=csb["ones1"][:], rhs=vp_row[:], start=True, stop=True)
                cp0 = wp.tile([P, S], BF16, tag="cp0")
                cp1 = wp.tile([P, S], BF16, tag="cp1")
                nc.vector.tensor_scalar(out=cp0[:], in0=vp_ps[:], scalar1=csb["bndp"][:, 0:1], scalar2=None, op0=A.is_gt)
                nc.vector.tensor_scalar(out=cp1[:], in0=vp_ps[:], scalar1=csb["bndp"][:, 1:2], scalar2=None, op0=A.is_gt)
                ve_ps = pb.tile([P, S], F32, tag="vps")
                nc.tensor.matmul(out=ve_ps[:], lhsT=csb["ones1"][:], rhs=ve_row[:], start=True, stop=True)
                ce0 = wp.tile([P, S], BF16, tag="ce0")
                ce1 = wp.tile([P, S], BF16, tag="ce1")
                nc.vector.tensor_scalar(out=ce0[:], in0=ve_ps[:], scalar1=csb["bnde"][:, 0:1], scalar2=None, op0=A.is_gt)
                nc.vector.tensor_scalar(out=ce1[:], in0=ve_ps[:], scalar1=csb["bnde"][:, 1:2], scalar2=None, op0=A.is_gt)

                # ---- y = enc + ptab[pbin] + etab[ebin] (bf16 scratch rows) ----
                y_sb = wp.tile([P, NCH, H], BF16, tag="y")
                for c in range(NCH):
                    eps = pe.tile([P, H], F32, tag="eps")
                    nc.tensor.matmul(out=eps[:], lhsT=cp0[:, ts(c, P)], rhs=csb["dpt"][:, 0, :], start=True, stop=False)
                    nc.tensor.matmul(out=eps[:], lhsT=cp1[:, ts(c, P)], rhs=csb["dpt"][:, 1, :], start=False, stop=False)
                    nc.tensor.matmul(out=eps[:], lhsT=ce0[:, ts(c, P)], rhs=csb["det"][:, 0, :], start=False, stop=False)
                    nc.tensor.matmul(out=eps[:], lhsT=ce1[:, ts(c, P)], rhs=csb["det"][:, 1, :], start=False, stop=False)
                    nc.tensor.matmul(out=eps[:], lhsT=csb["ones1"][:], rhs=csb["base"][:], start=False, stop=True)
                    nc.vector.tensor_tensor(out=y_sb[:, c, :], in0=eps[:], in1=enc_sb[:, c, :], op=A.add)

                # ---- y rows (+ zero padding rows) to HBM scratch ----
                nc.sync.dma_start(
                    out=y_dr[b * YROWS : b * YROWS + S, :].rearrange(
                        "(c p) f -> p c f", p=P
                    ),
                    in_=y_sb[:],
                )
                nc.sync.dma_start(
                    out=y_dr[b * YROWS + S : (b + 1) * YROWS, :], in_=zt[:]
                )

            def phase2(b):
                # ---- frame_idx = inclusive prefix of delta, 16-wrap layout ----
                d16 = wp.tile([16, T // 16], F32, tag="d16")
                nc.sync.dma_start(
                    out=d16[:],
                    in_=delta_dr[b][0:T, :].rearrange("(s p) o -> p (s o)", p=16),
                )
                # within-column (16 consecutive frames) inclusive prefix
                fi_ps = prp.tile([16, T // 16], F32, tag="rep")
                nc.tensor.matmul(out=fi_ps[:], lhsT=csb["lt16"][:], rhs=d16[:], start=True, stop=True)
                fi_sb = wp.tile([16, T // 16], F32, tag="fis")
                nc.vector.tensor_copy(out=fi_sb[:], in_=fi_ps[:])
                # column sums and their exclusive prefix (two halves of 128 cols)
                cs_sb = wp.tile([P, 2], F32, tag="cs")
                for hf in range(2):
                    cs_ps = pmi.tile([P, 1], F32, tag="micro")
                    nc.tensor.matmul(out=cs_ps[:], lhsT=d16[:, ts(hf, P)], rhs=csb["onescol16"][:], start=True, stop=True)
                    nc.vector.tensor_copy(out=cs_sb[:, hf : hf + 1], in_=cs_ps[:])
                cs0b_sb = wp.tile([P, P], F32, tag="cs0b")
                nc.vector.tensor_copy(out=cs0b_sb[:], in_=cs_sb[:, 0:1].to_broadcast([P, P]))
                cpfx_row = wp.tile([1, T // 16], F32, tag="cpfx")
                for hf in range(2):
                    ep_ps = pmi.tile([P, 1], F32, tag="micro")
                    nc.tensor.matmul(out=ep_ps[:], lhsT=csb["slt128"][:], rhs=cs_sb[:, hf : hf + 1], start=True, stop=hf == 0)
                    if hf == 1:
                        nc.tensor.matmul(out=ep_ps[:], lhsT=cs0b_sb[:], rhs=csb["onecol"][:, 0:1], start=False, stop=True)
                    ep_sb = wp.tile([P, 1], F32, tag="ep")
                    nc.vector.tensor_copy(out=ep_sb[:], in_=ep_ps[:])
                    tr_ps = pmi.tile([1, P], F32, tag="micro")
                    nc.tensor.transpose(out=tr_ps[:], in_=ep_sb[:], identity=csb["ident"][:])
                    nc.vector.tensor_copy(out=cpfx_row[:, ts(hf, P)], in_=tr_ps[:])

                # ---- replicate to 128 partitions + add column offsets; int16 ----
                rep_ps = prp.tile([P, T // 16], F32, tag="rep")
                nc.tensor.matmul(out=rep_ps[:], lhsT=csb["rep16"][:], rhs=fi_sb[:], start=True, stop=False)
                nc.tensor.matmul(out=rep_ps[:], lhsT=csb["ones1"][:], rhs=cpfx_row[:], start=False, stop=True)
                idx16 = ip.tile([P, T // 16], I16, tag=f"idx{b}")
                nc.vector.tensor_copy(out=idx16[:], in_=rep_ps[:])
                idx_tiles[b] = idx16

            def phase3(b, g, q):
                # ---- gather bf16 frames from y scratch; row 512 = zeros ----
                idx16 = idx_tiles[b]
                ysrc = y_dr[b * YROWS : (b + 1) * YROWS, :]
                g_sb = gp.tile([P, GCHUNK // P, H], BF16, tag="g")
                nc.vector.memset(g_sb[:], 0.0)
                dma_sem = nc.alloc_semaphore(f"gat_{b}_{g}")
                nc.gpsimd.dma_gather(
                    out_ap=g_sb[:],
                    in_ap=ysrc,
                    idxs_ap=idx16[:, g * (GCHUNK // 16) : (g + 1) * (GCHUNK // 16)],
                    num_idxs=GCHUNK,
                    num_idxs_reg=GCHUNK,
                    elem_size=H,
                    prepare_only=True,
                    sem=dma_sem,
                    queue_num=q,
                )
                nc.gpsimd.trigger_dma(count=None, queue_num=q)
                f_sb = op.tile([P, GCHUNK // P, H], F32, tag="f")
                nc.scalar.wait_ge(dma_sem, 16)
                nc.scalar.activation(out=f_sb[:], in_=g_sb[:], func=ACT_COPY)
                nc.scalar.dma_start(
                    out=out_dr[b][g * GCHUNK : (g + 1) * GCHUNK, :].rearrange(
                        "(c p) f -> p c f", p=P
                    ),
                    in_=f_sb[:],
                )

            for b in range(BPC):
                phase0(b)
            q = 0
            for b in range(BPC):
                phase1(b)
                phase2(b)
                for g in range(NGATHER):
                    phase3(b, g, q % NQ)
                    q += 1

    nc.compile()
    return nc


_NC_CACHE = {}


def _get_nc():
    if "nc" not in _NC_CACHE:
        _NC_CACHE["nc"] = build_nc()
    return _NC_CACHE["nc"]


def make_in_maps(inputs):
    enc = np.ascontiguousarray(np.asarray(inputs["encoder_output"], np.float32))
    pit = np.ascontiguousarray(np.asarray(inputs["pitch_target"], np.float32))
    ene = np.ascontiguousarray(np.asarray(inputs["energy_target"], np.float32))
    dur = np.ascontiguousarray(np.asarray(inputs["duration_target"], np.float32))
    ptab = np.asarray(inputs["pitch_table"], np.float32)
    etab = np.asarray(inputs["energy_table"], np.float32)
    consts = _host_constants(ptab, etab)
    in_maps = []
    for c in range(NCORES):
        sl = slice(c * BPC, (c + 1) * BPC)
        m = dict(consts)
        m["enc"] = enc[sl]
        m["pitch"] = pit[sl]
        m["energy"] = ene[sl]
        m["durt"] = dur[sl]
        in_maps.append(m)
    return in_maps


def run(inputs, trace=False):
    nc = _get_nc()
    in_maps = make_in_maps(inputs)
    res = run_bass_kernel_spmd(nc, in_maps, list(range(NCORES)), trace=trace)
    out = np.empty((B, T, H), np.float32)
    for c in range(NCORES):
        for b in range(BPC):
            out[c * BPC + b] = res.results[c][f"out{b}"]
    return out, res


def kernel(**inputs):
    out, _ = run(inputs, trace=False)
    return out


# revision 18
# speedup vs baseline: 1.1355x; 1.1355x over previous
"""Trainium2 Bass kernel for AccentVarianceAdaptor.

Computation (per batch row):
  pbin = searchsorted(linspace(50,400,256), clip(pitch,50,400), 'left')
  ebin = searchsorted(linspace(0,1,256),  clip(energy,0,1),  'left')
  y    = encoder + ptab[pbin] + etab[ebin]               # [S, H]
  dur  = max(round(duration), 1); cum = cumsum(dur)
  out[t] = y[searchsorted(cum, t, 'right')] * (t < cum[-1])  # [T, H]

Mapping to the hardware (one NeuronCore handles 4 batch rows):
  - table lookup: C[bin, tok] = (boundary[bin] < v[tok]) built with a K=1
    broadcast matmul + tensor_scalar(is_gt); then
    y = base + sum_half C_half.T @ dTab_half + encoder, where
    dTab[i] = tab[i+1] - tab[i] in bf16 (telescoping sum == row select).
  - durations: (d + 2^23) - 2^23 rounds half-to-even exactly in f32;
    cumulative sums via triangular matmuls with a PE-transpose supplying the
    inter-chunk offsets.
  - length-regulate: scatter 1.0 at delta[cum[j]] (indirect DMA); frame_idx =
    inclusive-prefix-sum(delta) via triangular matmuls in a 16-partition
    layout matching dma_gather's index format; dma_gather pulls bf16 y rows
    from HBM scratch (row 512 = zeros covers the ragged tail) using
    prepare_only descriptors spread over 4 SWDGE queues so transfers overlap;
    the scalar engine upcasts bf16->f32 and plain DMA stores the frames.
"""

import os
import sys

for _p in ("/opt/trn_rl_repo", "/root/.axon_site/_ro/trn_rl_repo"):
    if os.path.isdir(_p) and _p not in sys.path:
        sys.path.insert(0, _p)

import numpy as np

from concourse import bacc, mybir, tile
from concourse.bass import AP, IndirectOffsetOnAxis, ts
from concourse.bass_utils import run_bass_kernel_spmd

B, S, H = 32, 512, 256
NBINS = 256
T = 4096
NCORES = 8
BPC = B // NCORES  # batches per core
P = 128
NCH = S // P  # token chunks per batch
YROWS = S + 8  # y scratch rows per batch (512 tokens + zero rows)
DELTA_N = T + 8
GCHUNK = 1024  # max indices per dma_gather (SWDGE ring limit)
NGATHER = T // GCHUNK
NQ = 4  # SWDGE queues
F32 = mybir.dt.float32
BF16 = mybir.dt.bfloat16
I32 = mybir.dt.int32
I16 = mybir.dt.int16
A = mybir.AluOpType
ACT_COPY = mybir.ActivationFunctionType.Copy


def _boundaries():
    """Bit-exact copies of the f32 boundaries the jax reference uses."""
    import jax

    with jax.default_device(jax.devices("cpu")[0]):
        import jax.numpy as jnp

        bp = np.asarray(jnp.linspace(50.0, 400.0, NBINS), np.float32)
        be = np.asarray(jnp.linspace(0.0, 1.0, NBINS), np.float32)
    return bp, be


def _host_constants(pitch_table, energy_table):
    bp, be = _boundaries()
    consts = {}
    import ml_dtypes
    for name, tab in (("dpt", pitch_table), ("det", energy_table)):
        d = np.zeros((NBINS, H), np.float32)
        d[:-1] = tab[1:] - tab[:-1]  # f32 arithmetic, row 255 stays 0
        consts[name] = d.astype(ml_dtypes.bfloat16)
    consts["base"] = (pitch_table[0] + energy_table[0]).reshape(1, H)
    consts["bndp"] = bp.reshape(2, P).T.copy()  # [128, 2], col h = b[h*128 + p]
    consts["bnde"] = be.reshape(2, P).T.copy()
    j = np.arange(P, dtype=np.float32)
    consts["lt128"] = (j[:, None] <= j[None, :]).astype(np.float32)  # incl prefix
    consts["slt128"] = (j[:, None] < j[None, :]).astype(np.float32)  # excl prefix
    c4 = np.arange(NCH, dtype=np.float32)
    consts["slt4"] = (c4[:, None] < c4[None, :]).astype(np.float32)
    j16 = np.arange(16, dtype=np.float32)
    consts["lt16"] = (j16[:, None] <= j16[None, :]).astype(np.float32)
    consts["ones1"] = np.ones((1, P), np.float32)
    consts["ones1_16"] = np.ones((1, 16), np.float32)
    consts["onescol16"] = np.ones((16, 1), np.float32)
    consts["onecol"] = np.ones((P, 1), np.float32)
    consts["ones4"] = np.ones((P, NCH), np.float32)
    consts["ident"] = np.eye(P, dtype=np.float32)
    m = np.arange(P)
    consts["rep16"] = (m[None, :] % 16 == np.arange(16)[:, None]).astype(np.float32)
    consts["goff"] = (np.arange(T // GCHUNK, dtype=np.float32) * GCHUNK).reshape(1, -1)
    thr = np.full((P, T // 16), 511.5, np.float32)
    for g in range(T // GCHUNK):
        thr[0::16, g * (GCHUNK // 16)] = 1e9  # frame g*GCHUNK stays non-negative
    consts["thr"] = thr
    return consts


def build_nc():
    nc = bacc.Bacc(
        "TRN2",
        target_bir_lowering=False,
        debug=False,
        enable_asserts=False,
        num_swdge_queues=NQ,
    )

    enc_dr = nc.dram_tensor("enc", [BPC, S, H], F32, kind="ExternalInput")
    pit_dr = nc.dram_tensor("pitch", [BPC, S], F32, kind="ExternalInput")
    ene_dr = nc.dram_tensor("energy", [BPC, S], F32, kind="ExternalInput")
    dur_dr = nc.dram_tensor("durt", [BPC, S], F32, kind="ExternalInput")
    tab_dr = {
        nm: nc.dram_tensor(nm, [NBINS, H], BF16, kind="ExternalInput")
        for nm in ("dpt", "det")
    }
    cdr = {
        name: nc.dram_tensor(name, list(arr_shape), F32, kind="ExternalInput")
        for name, arr_shape in (
            ("base", (1, H)),
            ("bndp", (P, 2)),
            ("bnde", (P, 2)),
            ("lt128", (P, P)),
            ("slt128", (P, P)),
            ("slt4", (NCH, NCH)),
            ("lt16", (16, 16)),
            ("ones1", (1, P)),
            ("ones1_16", (1, 16)),
            ("onescol16", (16, 1)),
            ("onecol", (P, 1)),
            ("ones4", (P, NCH)),
            ("ident", (P, P)),
            ("rep16", (16, P)),
            ("goff", (1, NGATHER)),
            ("thr", (P, T // 16)),
        )
    }
    out_dr = [
        nc.dram_tensor(f"out{b}", [T, H], F32, kind="ExternalOutput")
        for b in range(BPC)
    ]
    y_dr = nc.dram_tensor("y_scr", [BPC * YROWS, H], BF16)
    delta_dr = [nc.dram_tensor(f"delta{b}", [DELTA_N, 1], F32) for b in range(BPC)]

    with tile.TileContext(nc) as tc:
        with (
            tc.tile_pool(name="const", bufs=1) as cp,
            tc.tile_pool(name="work", bufs=4) as wp,
            tc.tile_pool(name="gat", bufs=4) as gp,
            tc.tile_pool(name="gf32", bufs=4) as op,
            tc.tile_pool(name="idxp", bufs=BPC) as ip,
            tc.tile_pool(name="pbig", bufs=1, space="PSUM") as pb,
            tc.tile_pool(name="peps", bufs=2, space="PSUM") as pe,
            tc.tile_pool(name="psmall", bufs=2, space="PSUM") as psm,
            tc.tile_pool(name="pmicro", bufs=1, space="PSUM") as pmi,
            tc.tile_pool(name="prep", bufs=2, space="PSUM") as prp,
        ):
            # ---- constants ----
            csb = {}
            for name, dr in cdr.items():
                t_ = cp.tile(list(dr.shape), F32, tag=name)
                nc.sync.dma_start(out=t_[:], in_=dr[:])
                csb[name] = t_
            for nm, dr in tab_dr.items():
                t_ = cp.tile([P, 2, H], BF16, tag=nm)
                nc.sync.dma_start(
                    out=t_[:], in_=dr[:].rearrange("(h p) f -> p h f", p=P)
                )
                csb[nm] = t_
            zt = cp.tile([8, H], BF16)
            nc.gpsimd.memset(zt[:], 0.0)
            zrow = cp.tile([1, DELTA_N], F32)
            nc.gpsimd.memset(zrow[:], 0.0)

            idx_tiles = {}
            cnt_tiles = {}

            def phase0(b):
                # ---- dur load + frame-delta chain (tiny, unblocks gathers) ----
                dur_raw = wp.tile([P, NCH], F32, tag="draw")
                nc.sync.dma_start(
                    out=dur_raw[:], in_=dur_dr[b].rearrange("(c p) -> p c", p=P)
                )

                # ---- dur = max(round_half_even(durt), 1) ----
                MAGIC = float(1 << 23)
                dr0 = wp.tile([P, NCH], F32, tag="dr0")
                nc.vector.tensor_scalar(out=dr0[:], in0=dur_raw[:], scalar1=MAGIC, scalar2=MAGIC, op0=A.add, op1=A.subtract)
                dur_sb = wp.tile([P, NCH], F32, tag="dur")
                nc.vector.tensor_scalar(out=dur_sb[:], in0=dr0[:], scalar1=1.0, scalar2=None, op0=A.max)

                # ---- inclusive cum over tokens (wrapped j = pc*128 + p) ----
                i1_ps = psm.tile([P, NCH], F32, tag="small")
                nc.tensor.matmul(out=i1_ps[:], lhsT=csb["lt128"][:], rhs=dur_sb[:], start=True, stop=True)
                i1_sb = wp.tile([P, NCH], F32, tag="i1")
                nc.vector.tensor_copy(out=i1_sb[:], in_=i1_ps[:])
                tot_ps = psm.tile([NCH, P], F32, tag="small")
                nc.tensor.transpose(out=tot_ps[:], in_=i1_sb[:], identity=csb["ident"][:])
                tot_sb = wp.tile([NCH, P], F32, tag="tot")
                nc.vector.tensor_copy(out=tot_sb[:], in_=tot_ps[:])
                totb_sb = wp.tile([NCH, P], F32, tag="totb")
                nc.vector.tensor_copy(out=totb_sb[:], in_=tot_sb[:, P - 1 : P].to_broadcast([NCH, P]))
                cum_ps = psm.tile([P, NCH], F32, tag="small")
                nc.tensor.matmul(out=cum_ps[:], lhsT=csb["lt128"][:], rhs=dur_sb[:], start=True, stop=False)
                nc.tensor.matmul(out=cum_ps[:], lhsT=totb_sb[:], rhs=csb["slt4"][:], start=False, stop=True)
                cum_i32 = wp.tile([P, NCH], I32, tag="cumi")
                nc.vector.tensor_copy(out=cum_i32[:], in_=cum_ps[:])
                tot_row_ps = pmi.tile([1, NCH], F32, tag="micro")
                nc.tensor.matmul(out=tot_row_ps[:], lhsT=csb["onecol"][:], rhs=dur_sb[:], start=True, stop=True)
                tot1 = wp.tile([1, 1], F32, tag="tot1")
                nc.vector.tensor_reduce(out=tot1[:], in_=tot_row_ps[:], axis=mybir.AxisListType.X, op=A.add)
                cnt_f = wp.tile([1, NGATHER], F32, tag="cntf")
                nc.vector.tensor_tensor(
                    out=cnt_f[:],
                    in0=tot1[:].to_broadcast([1, NGATHER]),
                    in1=csb["goff"][:],
                    op=A.subtract,
                )
                cnt_i = wp.tile([1, NGATHER], I32, tag="cnti")
                nc.vector.tensor_scalar(out=cnt_i[:], in0=cnt_f[:], scalar1=1.0, scalar2=float(GCHUNK), op0=A.max, op1=A.min)
                cnt_tiles[b] = cnt_i

                # ---- delta: zero then scatter 1.0 at cum positions ----
                nc.sync.dma_start(
                    out=delta_dr[b][:].rearrange("v o -> o v"), in_=zrow[:]
                )
                for c in range(NCH):
                    nc.gpsimd.indirect_dma_start(
                        out=delta_dr[b][:],
                        out_offset=IndirectOffsetOnAxis(ap=cum_i32[:, c : c + 1], axis=0),
                        in_=csb["onecol"][:],
                        in_offset=None,
                    )

            def phase1(b):
                # ---- heavy loads + variance-adder compute ----
                enc_sb = wp.tile([P, NCH, H], F32, tag="enc")
                nc.sync.dma_start(
                    out=enc_sb[:], in_=enc_dr[b].rearrange("(c p) f -> p c f", p=P)
                )
                vp_row = wp.tile([1, S], F32, tag="vp")
                ve_row = wp.tile([1, S], F32, tag="ve")
                nc.sync.dma_start(out=vp_row[:], in_=pit_dr[b][None, :])
                nc.sync.dma_start(out=ve_row[:], in_=ene_dr[b][None, :])

                # ---- C matrices: C[bin_p, tok] = (boundary[bin] < v[tok]) ----
                vp_ps = pb.tile([P, S], F32, tag="vps")
                nc.tensor.matmul(out=vp_ps[:], lhsT=csb["ones1"][:], rhs=vp_row[:], start=True, stop=True)
                cp0 = wp.tile([P, S], BF16, tag="cp0")
                cp1 = wp.tile([P, S], BF16, tag="cp1")
                nc.vector.tensor_scalar(out=cp0[:], in0=vp_ps[:], scalar1=csb["bndp"][:, 0:1], scalar2=None, op0=A.is_gt)
                nc.vector.tensor_scalar(out=cp1[:], in0=vp_ps[:], scalar1=csb["bndp"][:, 1:2], scalar2=None, op0=A.is_gt)
                ve_ps = pb.tile([P, S], F32, tag="vps")
                nc.tensor.matmul(out=ve_ps[:], lhsT=csb["ones1"][:], rhs=ve_row[:], start=True, stop=True)
                ce0 = wp.tile([P, S], BF16, tag="ce0")
                ce1 = wp.tile([P, S], BF16, tag="ce1")
                nc.vector.tensor_scalar(out=ce0[:], in0=ve_ps[:], scalar1=csb["bnde"][:, 0:1], scalar2=None, op0=A.is_gt)
                nc.vector.tensor_scalar(out=ce1[:], in0=ve_ps[:], scalar1=csb["bnde"][:, 1:2], scalar2=None, op0=A.is_gt)

                # ---- y = enc + ptab[pbin] + etab[ebin] (bf16 scratch rows) ----
                y_sb = wp.tile([P, NCH, H], BF16, tag="y")
                for c in range(NCH):
                    eps = pe.tile([P, H], F32, tag="eps")
                    nc.tensor.matmul(out=eps[:], lhsT=cp0[:, ts(c, P)], rhs=csb["dpt"][:, 0, :], start=True, stop=False)
                    nc.tensor.matmul(out=eps[:], lhsT=cp1[:, ts(c, P)], rhs=csb["dpt"][:, 1, :], start=False, stop=False)
                    nc.tensor.matmul(out=eps[:], lhsT=ce0[:, ts(c, P)], rhs=csb["det"][:, 0, :], start=False, stop=False)
                    nc.tensor.matmul(out=eps[:], lhsT=ce1[:, ts(c, P)], rhs=csb["det"][:, 1, :], start=False, stop=False)
                    nc.tensor.matmul(out=eps[:], lhsT=csb["ones1"][:], rhs=csb["base"][:], start=False, stop=True)
                    nc.vector.tensor_tensor(out=y_sb[:, c, :], in0=eps[:], in1=enc_sb[:, c, :], op=A.add)

                # ---- y rows (+ zero padding rows) to HBM scratch ----
                nc.sync.dma_start(
                    out=y_dr[b * YROWS : b * YROWS + S, :].rearrange(
                        "(c p) f -> p c f", p=P
                    ),
                    in_=y_sb[:],
                )
                nc.sync.dma_start(
                    out=y_dr[b * YROWS + S : (b + 1) * YROWS, :], in_=zt[:]
                )

            def phase2(b):
                # ---- frame_idx = inclusive prefix of delta, 16-wrap layout ----
                d16 = wp.tile([16, T // 16], F32, tag="d16")
                nc.sync.dma_start(
                    out=d16[:],
                    in_=delta_dr[b][0:T, :].rearrange("(s p) o -> p (s o)", p=16),
                )
                # within-column (16 consecutive frames) inclusive prefix
                fi_ps = prp.tile([16, T // 16], F32, tag="rep")
                nc.tensor.matmul(out=fi_ps[:], lhsT=csb["lt16"][:], rhs=d16[:], start=True, stop=True)
                fi_sb = wp.tile([16, T // 16], F32, tag="fis")
                nc.vector.tensor_copy(out=fi_sb[:], in_=fi_ps[:])
                # column sums and their exclusive prefix (two halves of 128 cols)
                cs_sb = wp.tile([P, 2], F32, tag="cs")
                for hf in range(2):
                    cs_ps = pmi.tile([P, 1], F32, tag="micro")
                    nc.tensor.matmul(out=cs_ps[:], lhsT=d16[:, ts(hf, P)], rhs=csb["onescol16"][:], start=True, stop=True)
                    nc.vector.tensor_copy(out=cs_sb[:, hf : hf + 1], in_=cs_ps[:])
                cs0b_sb = wp.tile([P, P], F32, tag="cs0b")
                nc.vector.tensor_copy(out=cs0b_sb[:], in_=cs_sb[:, 0:1].to_broadcast([P, P]))
                cpfx_row = wp.tile([1, T // 16], F32, tag="cpfx")
                for hf in range(2):
                    ep_ps = pmi.tile([P, 1], F32, tag="micro")
                    nc.tensor.matmul(out=ep_ps[:], lhsT=csb["slt128"][:], rhs=cs_sb[:, hf : hf + 1], start=True, stop=hf == 0)
                    if hf == 1:
                        nc.tensor.matmul(out=ep_ps[:], lhsT=cs0b_sb[:], rhs=csb["onecol"][:, 0:1], start=False, stop=True)
                    ep_sb = wp.tile([P, 1], F32, tag="ep")
                    nc.vector.tensor_copy(out=ep_sb[:], in_=ep_ps[:])
                    tr_ps = pmi.tile([1, P], F32, tag="micro")
                    nc.tensor.transpose(out=tr_ps[:], in_=ep_sb[:], identity=csb["ident"][:])
                    nc.vector.tensor_copy(out=cpfx_row[:, ts(hf, P)], in_=tr_ps[:])

                # ---- replicate to 128 partitions + add column offsets; int16 ----
                rep_ps = prp.tile([P, T // 16], F32, tag="rep")
                nc.tensor.matmul(out=rep_ps[:], lhsT=csb["rep16"][:], rhs=fi_sb[:], start=True, stop=False)
                nc.tensor.matmul(out=rep_ps[:], lhsT=csb["ones1"][:], rhs=cpfx_row[:], start=False, stop=True)
                idx16 = ip.tile([P, T // 16], I16, tag=f"idx{b}")
                nc.vector.tensor_copy(out=idx16[:], in_=rep_ps[:])
                idx_tiles[b] = idx16

            def phase3(b, g, q):
                # ---- gather bf16 frames from y scratch; row 512 = zeros ----
                idx16 = idx_tiles[b]
                ysrc = y_dr[b * YROWS : (b + 1) * YROWS, :]
                g_sb = gp.tile([P, GCHUNK // P, H], BF16, tag="g")
                nc.vector.memset(g_sb[:], 0.0)
                dma_sem = nc.alloc_semaphore(f"gat_{b}_{g}")
                nc.gpsimd.dma_gather(
                    out_ap=g_sb[:],
                    in_ap=ysrc,
                    idxs_ap=idx16[:, g * (GCHUNK // 16) : (g + 1) * (GCHUNK // 16)],
                    num_idxs=GCHUNK,
                    num_idxs_reg=GCHUNK,
                    elem_size=H,
                    prepare_only=True,
                    sem=dma_sem,
                    queue_num=q,
                )
                nc.gpsimd.trigger_dma(count=None, queue_num=q)
                f_sb = op.tile([P, GCHUNK // P, H], F32, tag="f")
                nc.scalar.wait_ge(dma_sem, 16)
                nc.scalar.activation(out=f_sb[:], in_=g_sb[:], func=ACT_COPY)
                nc.scalar.dma_start(
                    out=out_dr[b][g * GCHUNK : (g + 1) * GCHUNK, :].rearrange(
                        "(c p) f -> p c f", p=P
                    ),
                    in_=f_sb[:],
                )

            for b in range(BPC):
                phase0(b)
            q = 0
            for b in range(BPC):
                phase1(b)
                phase2(b)
                for g in range(NGATHER):
                    phase3(b, g, q % NQ)
                    q += 1

    nc.compile()
    return nc


_NC_CACHE = {}


def _get_nc():
    if "nc" not in _NC_CACHE:
        _NC_CACHE["nc"] = build_nc()
    return _NC_CACHE["nc"]


def make_in_maps(inputs):
    enc = np.ascontiguousarray(np.asarray(inputs["encoder_output"], np.float32))
    pit = np.ascontiguousarray(np.asarray(inputs["pitch_target"], np.float32))
    ene = np.ascontiguousarray(np.asarray(inputs["energy_target"], np.float32))
    dur = np.ascontiguousarray(np.asarray(inputs["duration_target"], np.float32))
    ptab = np.asarray(inputs["pitch_table"], np.float32)
    etab = np.asarray(inputs["energy_table"], np.float32)
    consts = _host_constants(ptab, etab)
    in_maps = []
    for c in range(NCORES):
        sl = slice(c * BPC, (c + 1) * BPC)
        m = dict(consts)
        m["enc"] = enc[sl]
        m["pitch"] = pit[sl]
        m["energy"] = ene[sl]
        m["durt"] = dur[sl]
        in_maps.append(m)
    return in_maps


def run(inputs, trace=False):
    nc = _get_nc()
    in_maps = make_in_maps(inputs)
    res = run_bass_kernel_spmd(nc, in_maps, list(range(NCORES)), trace=trace)
    out = np.empty((B, T, H), np.float32)
    for c in range(NCORES):
        for b in range(BPC):
            out[c * BPC + b] = res.results[c][f"out{b}"]
    return out, res


def kernel(**inputs):
    out, _ = run(inputs, trace=False)
    return out


# revision 22
# speedup vs baseline: 1.3992x; 1.2323x over previous
"""Trainium2 Bass kernel for AccentVarianceAdaptor.

Computation (per batch row):
  pbin = searchsorted(linspace(50,400,256), clip(pitch,50,400), 'left')
  ebin = searchsorted(linspace(0,1,256),  clip(energy,0,1),  'left')
  y    = encoder + ptab[pbin] + etab[ebin]               # [S, H]
  dur  = max(round(duration), 1); cum = cumsum(dur)
  out[t] = y[searchsorted(cum, t, 'right')] * (t < cum[-1])  # [T, H]

Mapping to the hardware (one NeuronCore handles 4 batch rows):
  - table lookup: C[bin, tok] = (boundary[bin] < v[tok]) built with a K=1
    broadcast matmul + tensor_scalar(is_gt); then
    y = base + sum_half C_half.T @ dTab_half + encoder, where
    dTab[i] = tab[i+1] - tab[i] in bf16 (telescoping sum == row select).
  - durations: (d + 2^23) - 2^23 rounds half-to-even exactly in f32;
    cumulative sums via triangular matmuls with a PE-transpose supplying the
    inter-chunk offsets.
  - length-regulate: scatter 1.0 at delta[cum[j]] (indirect DMA); frame_idx =
    inclusive-prefix-sum(delta) via triangular matmuls in a 16-partition
    layout matching dma_gather's index format; dma_gather pulls bf16 y rows
    from HBM scratch (row 512 = zeros covers the ragged tail) using
    prepare_only descriptors spread over 4 SWDGE queues so transfers overlap;
    the scalar engine upcasts bf16->f32 and plain DMA stores the frames.
"""

import os
import sys

for _p in ("/opt/trn_rl_repo", "/root/.axon_site/_ro/trn_rl_repo"):
    if os.path.isdir(_p) and _p not in sys.path:
        sys.path.insert(0, _p)

import numpy as np

from concourse import bacc, mybir, tile
from concourse.bass import AP, IndirectOffsetOnAxis, ts
from concourse.bass_utils import run_bass_kernel_spmd

B, S, H = 32, 512, 256
NBINS = 256
T = 4096
NCORES = 8
BPC = B // NCORES  # batches per core
P = 128
NCH = S // P  # token chunks per batch
YROWS = S + 8  # y scratch rows per batch (512 tokens + zero rows)
DELTA_N = T + 8
GCHUNK = 1024  # max indices per dma_gather (SWDGE ring limit)
NPAIR = T // 2  # frame pairs per batch
NGATHER = NPAIR // GCHUNK
PTROWS = 1026  # pair-table rows per batch
NQ = 4  # SWDGE queues
F32 = mybir.dt.float32
BF16 = mybir.dt.bfloat16
I32 = mybir.dt.int32
I16 = mybir.dt.int16
A = mybir.AluOpType
ACT_COPY = mybir.ActivationFunctionType.Copy


def _boundaries():
    """Bit-exact copies of the f32 boundaries the jax reference uses."""
    import jax

    with jax.default_device(jax.devices("cpu")[0]):
        import jax.numpy as jnp

        bp = np.asarray(jnp.linspace(50.0, 400.0, NBINS), np.float32)
        be = np.asarray(jnp.linspace(0.0, 1.0, NBINS), np.float32)
    return bp, be


def _host_constants(pitch_table, energy_table):
    bp, be = _boundaries()
    consts = {}
    import ml_dtypes
    for name, tab in (("dpt", pitch_table), ("det", energy_table)):
        d = np.zeros((NBINS, H), np.float32)
        d[:-1] = tab[1:] - tab[:-1]  # f32 arithmetic, row 255 stays 0
        consts[name] = d.astype(ml_dtypes.bfloat16)
    consts["base"] = (pitch_table[0] + energy_table[0]).reshape(1, H)
    consts["bndp"] = bp.reshape(2, P).T.copy()  # [128, 2], col h = b[h*128 + p]
    consts["bnde"] = be.reshape(2, P).T.copy()
    j = np.arange(P, dtype=np.float32)
    consts["lt128"] = (j[:, None] <= j[None, :]).astype(np.float32)  # incl prefix
    consts["slt128"] = (j[:, None] < j[None, :]).astype(np.float32)  # excl prefix
    c4 = np.arange(NCH, dtype=np.float32)
    consts["slt4"] = (c4[:, None] < c4[None, :]).astype(np.float32)
    j16 = np.arange(16, dtype=np.float32)
    consts["lt16"] = (j16[:, None] <= j16[None, :]).astype(np.float32)
    consts["ones1"] = np.ones((1, P), np.float32)
    consts["ones1_16"] = np.ones((1, 16), np.float32)
    consts["onescol16"] = np.ones((16, 1), np.float32)
    consts["onecol"] = np.ones((P, 1), np.float32)
    consts["ones4"] = np.ones((P, NCH), np.float32)
    consts["ident"] = np.eye(P, dtype=np.float32)
    m = np.arange(P)
    consts["rep16"] = (m[None, :] % 16 == np.arange(16)[:, None]).astype(np.float32)
    m16 = np.arange(16)
    consts["nrep16"] = (-514.0 * (m[None, :] % 16 == m16[:, None])).astype(np.float32)
    return consts


def build_nc():
    nc = bacc.Bacc(
        "TRN2",
        target_bir_lowering=False,
        debug=False,
        enable_asserts=False,
        num_swdge_queues=NQ,
    )

    enc_dr = nc.dram_tensor("enc", [BPC, S, H], F32, kind="ExternalInput")
    pit_dr = nc.dram_tensor("pitch", [BPC, S], F32, kind="ExternalInput")
    ene_dr = nc.dram_tensor("energy", [BPC, S], F32, kind="ExternalInput")
    dur_dr = nc.dram_tensor("durt", [BPC, S], F32, kind="ExternalInput")
    tab_dr = {
        nm: nc.dram_tensor(nm, [NBINS, H], BF16, kind="ExternalInput")
        for nm in ("dpt", "det")
    }
    cdr = {
        name: nc.dram_tensor(name, list(arr_shape), F32, kind="ExternalInput")
        for name, arr_shape in (
            ("base", (1, H)),
            ("bndp", (P, 2)),
            ("bnde", (P, 2)),
            ("lt128", (P, P)),
            ("slt128", (P, P)),
            ("slt4", (NCH, NCH)),
            ("lt16", (16, 16)),
            ("ones1", (1, P)),
            ("ones1_16", (1, 16)),
            ("onescol16", (16, 1)),
            ("onecol", (P, 1)),
            ("ones4", (P, NCH)),
            ("ident", (P, P)),
            ("rep16", (16, P)),
            ("nrep16", (16, P)),
        )
    }
    out_dr = [
        nc.dram_tensor(f"out{b}", [T, H], F32, kind="ExternalOutput")
        for b in range(BPC)
    ]
    y_dr = nc.dram_tensor("y_scr", [BPC * YROWS, H], BF16)
    # pair table rows: [0..512] = y2 (y[j], y[j+1]); [513..1025] = ydup (y[j], y[j])
    y2_dr = nc.dram_tensor("y2_scr", [BPC * PTROWS, 2 * H], BF16)
    delta_dr = [nc.dram_tensor(f"delta{b}", [DELTA_N, 1], F32) for b in range(BPC)]

    with tile.TileContext(nc) as tc:
        with (
            tc.tile_pool(name="const", bufs=1) as cp,
            tc.tile_pool(name="work", bufs=4) as wp,
            tc.tile_pool(name="gat", bufs=4) as gp,
            tc.tile_pool(name="gf32", bufs=4) as op,
            tc.tile_pool(name="idxp", bufs=BPC) as ip,
            tc.tile_pool(name="pbig", bufs=1, space="PSUM") as pb,
            tc.tile_pool(name="peps", bufs=2, space="PSUM") as pe,
            tc.tile_pool(name="psmall", bufs=2, space="PSUM") as psm,
            tc.tile_pool(name="pmicro", bufs=1, space="PSUM") as pmi,
            tc.tile_pool(name="prep", bufs=2, space="PSUM") as prp,
        ):
            # ---- constants ----
            csb = {}
            for name, dr in cdr.items():
                t_ = cp.tile(list(dr.shape), F32, tag=name)
                nc.sync.dma_start(out=t_[:], in_=dr[:])
                csb[name] = t_
            for nm, dr in tab_dr.items():
                t_ = cp.tile([P, 2, H], BF16, tag=nm)
                nc.sync.dma_start(
                    out=t_[:], in_=dr[:].rearrange("(h p) f -> p h f", p=P)
                )
                csb[nm] = t_
            zt = cp.tile([8, H], BF16)
            nc.gpsimd.memset(zt[:], 0.0)
            zrow = cp.tile([1, DELTA_N], F32)
            nc.gpsimd.memset(zrow[:], 0.0)

            idx_tiles = {}

            def phase0(b):
                # ---- dur load + frame-delta chain (tiny, unblocks gathers) ----
                dur_raw = wp.tile([P, NCH], F32, tag="draw")
                nc.sync.dma_start(
                    out=dur_raw[:], in_=dur_dr[b].rearrange("(c p) -> p c", p=P)
                )

                # ---- dur = max(round_half_even(durt), 1) ----
                MAGIC = float(1 << 23)
                dr0 = wp.tile([P, NCH], F32, tag="dr0")
                nc.vector.tensor_scalar(out=dr0[:], in0=dur_raw[:], scalar1=MAGIC, scalar2=MAGIC, op0=A.add, op1=A.subtract)
                dur_sb = wp.tile([P, NCH], F32, tag="dur")
                nc.vector.tensor_scalar(out=dur_sb[:], in0=dr0[:], scalar1=1.0, scalar2=None, op0=A.max)

                # ---- inclusive cum over tokens (wrapped j = pc*128 + p) ----
                i1_ps = psm.tile([P, NCH], F32, tag="small")
                nc.tensor.matmul(out=i1_ps[:], lhsT=csb["lt128"][:], rhs=dur_sb[:], start=True, stop=True)
                i1_sb = wp.tile([P, NCH], F32, tag="i1")
                nc.vector.tensor_copy(out=i1_sb[:], in_=i1_ps[:])
                tot_ps = psm.tile([NCH, P], F32, tag="small")
                nc.tensor.transpose(out=tot_ps[:], in_=i1_sb[:], identity=csb["ident"][:])
                tot_sb = wp.tile([NCH, P], F32, tag="tot")
                nc.vector.tensor_copy(out=tot_sb[:], in_=tot_ps[:])
                totb_sb = wp.tile([NCH, P], F32, tag="totb")
                nc.vector.tensor_copy(out=totb_sb[:], in_=tot_sb[:, P - 1 : P].to_broadcast([NCH, P]))
                cum_ps = psm.tile([P, NCH], F32, tag="small")
                nc.tensor.matmul(out=cum_ps[:], lhsT=csb["lt128"][:], rhs=dur_sb[:], start=True, stop=False)
                nc.tensor.matmul(out=cum_ps[:], lhsT=totb_sb[:], rhs=csb["slt4"][:], start=False, stop=True)
                cum_i32 = wp.tile([P, NCH], I32, tag="cumi")
                nc.vector.tensor_copy(out=cum_i32[:], in_=cum_ps[:])


                # ---- delta: zero then scatter 1.0 at cum positions ----
                nc.sync.dma_start(
                    out=delta_dr[b][:].rearrange("v o -> o v"), in_=zrow[:]
                )
                for c in range(NCH):
                    nc.gpsimd.indirect_dma_start(
                        out=delta_dr[b][:],
                        out_offset=IndirectOffsetOnAxis(ap=cum_i32[:, c : c + 1], axis=0),
                        in_=csb["onecol"][:],
                        in_offset=None,
                    )

            def phase1(b):
                # ---- heavy loads + variance-adder compute ----
                enc_sb = wp.tile([P, NCH, H], F32, tag="enc")
                nc.sync.dma_start(
                    out=enc_sb[:], in_=enc_dr[b].rearrange("(c p) f -> p c f", p=P)
                )
                vp_row = wp.tile([1, S], F32, tag="vp")
                ve_row = wp.tile([1, S], F32, tag="ve")
                nc.sync.dma_start(out=vp_row[:], in_=pit_dr[b][None, :])
                nc.sync.dma_start(out=ve_row[:], in_=ene_dr[b][None, :])

                # ---- C matrices: C[bin_p, tok] = (boundary[bin] < v[tok]) ----
                vp_ps = pb.tile([P, S], F32, tag="vps")
                nc.tensor.matmul(out=vp_ps[:], lhsT=csb["ones1"][:], rhs=vp_row[:], start=True, stop=True)
                cp0 = wp.tile([P, S], BF16, tag="cp0")
                cp1 = wp.tile([P, S], BF16, tag="cp1")
                nc.vector.tensor_scalar(out=cp0[:], in0=vp_ps[:], scalar1=csb["bndp"][:, 0:1], scalar2=None, op0=A.is_gt)
                nc.vector.tensor_scalar(out=cp1[:], in0=vp_ps[:], scalar1=csb["bndp"][:, 1:2], scalar2=None, op0=A.is_gt)
                ve_ps = pb.tile([P, S], F32, tag="vps")
                nc.tensor.matmul(out=ve_ps[:], lhsT=csb["ones1"][:], rhs=ve_row[:], start=True, stop=True)
                ce0 = wp.tile([P, S], BF16, tag="ce0")
                ce1 = wp.tile([P, S], BF16, tag="ce1")
                nc.vector.tensor_scalar(out=ce0[:], in0=ve_ps[:], scalar1=csb["bnde"][:, 0:1], scalar2=None, op0=A.is_gt)
                nc.vector.tensor_scalar(out=ce1[:], in0=ve_ps[:], scalar1=csb["bnde"][:, 1:2], scalar2=None, op0=A.is_gt)

                # ---- y = enc + ptab[pbin] + etab[ebin] (bf16 scratch rows) ----
                y_sb = wp.tile([P, NCH, H], BF16, tag="y")
                for c in range(NCH):
                    eps = pe.tile([P, H], F32, tag="eps")
                    nc.tensor.matmul(out=eps[:], lhsT=cp0[:, ts(c, P)], rhs=csb["dpt"][:, 0, :], start=True, stop=False)
                    nc.tensor.matmul(out=eps[:], lhsT=cp1[:, ts(c, P)], rhs=csb["dpt"][:, 1, :], start=False, stop=False)
                    nc.tensor.matmul(out=eps[:], lhsT=ce0[:, ts(c, P)], rhs=csb["det"][:, 0, :], start=False, stop=False)
                    nc.tensor.matmul(out=eps[:], lhsT=ce1[:, ts(c, P)], rhs=csb["det"][:, 1, :], start=False, stop=False)
                    nc.tensor.matmul(out=eps[:], lhsT=csb["ones1"][:], rhs=csb["base"][:], start=False, stop=True)
                    nc.vector.tensor_tensor(out=y_sb[:, c, :], in0=eps[:], in1=enc_sb[:, c, :], op=A.add)

                # ---- y rows (+ zero padding rows) to HBM scratch ----
                nc.sync.dma_start(
                    out=y_dr[b * YROWS : b * YROWS + S, :].rearrange(
                        "(c p) f -> p c f", p=P
                    ),
                    in_=y_sb[:],
                )
                nc.sync.dma_start(
                    out=y_dr[b * YROWS + S : (b + 1) * YROWS, :], in_=zt[:]
                )
                # pair tables (HBM->HBM; y row 512 is zeros so boundaries work out)
                yb = y_dr[b * YROWS : b * YROWS + S + 1, :]
                p2 = y2_dr[b * PTROWS : (b + 1) * PTROWS, :]
                nc.sync.dma_start(out=p2[0 : S + 1, 0:H], in_=yb[0 : S + 1, :])
                nc.sync.dma_start(out=p2[0:S, H : 2 * H], in_=yb[1 : S + 1, :])
                nc.sync.dma_start(out=p2[S : S + 1, H : 2 * H], in_=yb[S : S + 1, :])
                nc.sync.dma_start(out=p2[S + 1 : PTROWS, 0:H], in_=yb[0 : S + 1, :])
                nc.sync.dma_start(out=p2[S + 1 : PTROWS, H : 2 * H], in_=yb[0 : S + 1, :])

            def phase2(b):
                # ---- pair-space frame indexing: position k = s*16+p16 is the
                # frame pair (2k, 2k+1); dpair[p16, s, r] = delta[2k+r] ----
                NPC = NPAIR // 16  # pair columns (128)
                dpair = wp.tile([16, NPC, 2], F32, tag="d16")
                nc.sync.dma_start(
                    out=dpair[:],
                    in_=delta_dr[b][0:T, :].rearrange("(s p r) o -> p s (r o)", p=16, r=2),
                )
                ds = wp.tile([16, NPC], F32, tag="ds")
                nc.vector.tensor_tensor(out=ds[:], in0=dpair[:, :, 0], in1=dpair[:, :, 1], op=A.add)
                # within-column (16 consecutive pairs) inclusive prefix of ds
                fi_ps = prp.tile([16, NPC], F32, tag="rep")
                nc.tensor.matmul(out=fi_ps[:], lhsT=csb["lt16"][:], rhs=ds[:], start=True, stop=True)
                fi_sb = wp.tile([16, NPC], F32, tag="fis")
                nc.vector.tensor_copy(out=fi_sb[:], in_=fi_ps[:])
                # column sums and their exclusive prefix (128 cols)
                cs_ps = pmi.tile([P, 1], F32, tag="micro")
                nc.tensor.matmul(out=cs_ps[:], lhsT=ds[:], rhs=csb["onescol16"][:], start=True, stop=True)
                cs_sb = wp.tile([P, 1], F32, tag="cs")
                nc.vector.tensor_copy(out=cs_sb[:], in_=cs_ps[:])
                ep_ps = pmi.tile([P, 1], F32, tag="micro")
                nc.tensor.matmul(out=ep_ps[:], lhsT=csb["slt128"][:], rhs=cs_sb[:], start=True, stop=True)
                ep_sb = wp.tile([P, 1], F32, tag="ep")
                nc.vector.tensor_copy(out=ep_sb[:], in_=ep_ps[:])
                tr_ps = pmi.tile([1, P], F32, tag="micro")
                nc.tensor.transpose(out=tr_ps[:], in_=ep_sb[:], identity=csb["ident"][:])
                cpfx_row = wp.tile([1, NPC], F32, tag="cpfx")
                # +513 bias folds the ydup-table offset into the column prefix
                nc.vector.tensor_scalar(out=cpfx_row[:], in0=tr_ps[:], scalar1=513.0, scalar2=None, op0=A.add)

                # ---- table row index: P(k) + 513 - 514*delta[2k+1] ----
                rep_ps = prp.tile([P, NPC], F32, tag="rep")
                nc.tensor.matmul(out=rep_ps[:], lhsT=csb["rep16"][:], rhs=fi_sb[:], start=True, stop=False)
                nc.tensor.matmul(out=rep_ps[:], lhsT=csb["ones1"][:], rhs=cpfx_row[:], start=False, stop=False)
                nc.tensor.matmul(out=rep_ps[:], lhsT=csb["nrep16"][:], rhs=dpair[:, :, 1], start=False, stop=True)
                idx16 = ip.tile([P, NPC], I16, tag=f"idx{b}")
                nc.vector.tensor_copy(out=idx16[:], in_=rep_ps[:])
                idx_tiles[b] = idx16

            def phase3(b, g, q):
                # ---- gather bf16 frame-pairs from the pair table ----
                idx16 = idx_tiles[b]
                ysrc = y2_dr[b * PTROWS : (b + 1) * PTROWS, :]
                g_sb = gp.tile([P, GCHUNK // P, 2 * H], BF16, tag="g")
                dma_sem = nc.alloc_semaphore(f"gat_{b}_{g}")
                nc.gpsimd.dma_gather(
                    out_ap=g_sb[:],
                    in_ap=ysrc,
                    idxs_ap=idx16[:, g * (GCHUNK // 16) : (g + 1) * (GCHUNK // 16)],
                    num_idxs=GCHUNK,
                    num_idxs_reg=GCHUNK,
                    elem_size=2 * H,
                    prepare_only=True,
                    sem=dma_sem,
                    queue_num=q,
                )
                nc.gpsimd.trigger_dma(count=None, queue_num=q)
                f_sb = op.tile([P, GCHUNK // P, 2 * H], F32, tag="f")
                nc.scalar.wait_ge(dma_sem, 16)
                nc.scalar.activation(out=f_sb[:], in_=g_sb[:], func=ACT_COPY)
                nc.scalar.dma_start(
                    out=out_dr[b][g * 2 * GCHUNK : (g + 1) * 2 * GCHUNK, :].rearrange(
                        "(s p r) f -> p s (r f)", p=P, r=2
                    ),
                    in_=f_sb[:],
                )

            for b in range(BPC):
                phase0(b)
            q = 0
            for b in range(BPC):
                phase1(b)
                phase2(b)
                for g in range(NGATHER):
                    phase3(b, g, q % NQ)
                    q += 1

    nc.compile()
    return nc


_NC_CACHE = {}


def _get_nc():
    if "nc" not in _NC_CACHE:
        _NC_CACHE["nc"] = build_nc()
    return _NC_CACHE["nc"]


def make_in_maps(inputs):
    enc = np.ascontiguousarray(np.asarray(inputs["encoder_output"], np.float32))
    pit = np.ascontiguousarray(np.asarray(inputs["pitch_target"], np.float32))
    ene = np.ascontiguousarray(np.asarray(inputs["energy_target"], np.float32))
    dur = np.ascontiguousarray(np.asarray(inputs["duration_target"], np.float32))
    ptab = np.asarray(inputs["pitch_table"], np.float32)
    etab = np.asarray(inputs["energy_table"], np.float32)
    consts = _host_constants(ptab, etab)
    in_maps = []
    for c in range(NCORES):
        sl = slice(c * BPC, (c + 1) * BPC)
        m = dict(consts)
        m["enc"] = enc[sl]
        m["pitch"] = pit[sl]
        m["energy"] = ene[sl]
        m["durt"] = dur[sl]
        in_maps.append(m)
    return in_maps


def run(inputs, trace=False):
    nc = _get_nc()
    in_maps = make_in_maps(inputs)
    res = run_bass_kernel_spmd(nc, in_maps, list(range(NCORES)), trace=trace)
    out = np.empty((B, T, H), np.float32)
    for c in range(NCORES):
        for b in range(BPC):
            out[c * BPC + b] = res.results[c][f"out{b}"]
    return out, res


def kernel(**inputs):
    out, _ = run(inputs, trace=False)
    return out


# revision 24
# speedup vs baseline: 1.4029x; 1.0027x over previous
"""Trainium2 Bass kernel for AccentVarianceAdaptor.

Computation (per batch row):
  pbin = searchsorted(linspace(50,400,256), clip(pitch,50,400), 'left')
  ebin = searchsorted(linspace(0,1,256),  clip(energy,0,1),  'left')
  y    = encoder + ptab[pbin] + etab[ebin]               # [S, H]
  dur  = max(round(duration), 1); cum = cumsum(dur)
  out[t] = y[searchsorted(cum, t, 'right')] * (t < cum[-1])  # [T, H]

Mapping to the hardware (one NeuronCore handles 4 batch rows):
  - table lookup: C[bin, tok] = (boundary[bin] < v[tok]) built with a K=1
    broadcast matmul + tensor_scalar(is_gt); then
    y = base + sum_half C_half.T @ dTab_half + encoder, where
    dTab[i] = tab[i+1] - tab[i] in bf16 (telescoping sum == row select).
  - durations: (d + 2^23) - 2^23 rounds half-to-even exactly in f32;
    cumulative sums via triangular matmuls with a PE-transpose supplying the
    inter-chunk offsets.
  - length-regulate: scatter 1.0 at delta[cum[j]] (indirect DMA); frame_idx =
    inclusive-prefix-sum(delta) via triangular matmuls in a 16-partition
    layout matching dma_gather's index format; dma_gather pulls bf16 y rows
    from HBM scratch (row 512 = zeros covers the ragged tail) using
    prepare_only descriptors spread over 4 SWDGE queues so transfers overlap;
    the scalar engine upcasts bf16->f32 and plain DMA stores the frames.
"""

import os
import sys

for _p in ("/opt/trn_rl_repo", "/root/.axon_site/_ro/trn_rl_repo"):
    if os.path.isdir(_p) and _p not in sys.path:
        sys.path.insert(0, _p)

import numpy as np

from concourse import bacc, mybir, tile
from concourse.bass import AP, IndirectOffsetOnAxis, ts
from concourse.bass_utils import run_bass_kernel_spmd

B, S, H = 32, 512, 256
NBINS = 256
T = 4096
NCORES = 8
BPC = B // NCORES  # batches per core
P = 128
NCH = S // P  # token chunks per batch
YROWS = S + 8  # y scratch rows per batch (512 tokens + zero rows)
DELTA_N = T + 8
GCHUNK = 1024  # max indices per dma_gather (SWDGE ring limit)
NPAIR = T // 2  # frame pairs per batch
NGATHER = NPAIR // GCHUNK
PTU = 1028  # duplicated-row table units per batch (2*513 + pad)
NQ = 4  # SWDGE queues
F32 = mybir.dt.float32
BF16 = mybir.dt.bfloat16
I32 = mybir.dt.int32
I16 = mybir.dt.int16
A = mybir.AluOpType
ACT_COPY = mybir.ActivationFunctionType.Copy


def _boundaries():
    """Bit-exact copies of the f32 boundaries the jax reference uses."""
    import jax

    with jax.default_device(jax.devices("cpu")[0]):
        import jax.numpy as jnp

        bp = np.asarray(jnp.linspace(50.0, 400.0, NBINS), np.float32)
        be = np.asarray(jnp.linspace(0.0, 1.0, NBINS), np.float32)
    return bp, be


def _host_constants(pitch_table, energy_table):
    bp, be = _boundaries()
    consts = {}
    import ml_dtypes
    for name, tab in (("dpt", pitch_table), ("det", energy_table)):
        d = np.zeros((NBINS, H), np.float32)
        d[:-1] = tab[1:] - tab[:-1]  # f32 arithmetic, row 255 stays 0
        consts[name] = d.astype(ml_dtypes.bfloat16)
    consts["base"] = (pitch_table[0] + energy_table[0]).reshape(1, H)
    consts["bndp"] = bp.reshape(2, P).T.copy()  # [128, 2], col h = b[h*128 + p]
    consts["bnde"] = be.reshape(2, P).T.copy()
    j = np.arange(P, dtype=np.float32)
    consts["lt128"] = (j[:, None] <= j[None, :]).astype(np.float32)  # incl prefix
    consts["slt128"] = (j[:, None] < j[None, :]).astype(np.float32)  # excl prefix
    c4 = np.arange(NCH, dtype=np.float32)
    consts["slt4"] = (c4[:, None] < c4[None, :]).astype(np.float32)
    j16 = np.arange(16, dtype=np.float32)
    consts["lt16"] = (j16[:, None] <= j16[None, :]).astype(np.float32)
    consts["ones1"] = np.ones((1, P), np.float32)
    consts["ones1_16"] = np.ones((1, 16), np.float32)
    consts["onescol16"] = np.ones((16, 1), np.float32)
    consts["onecol"] = np.ones((P, 1), np.float32)
    consts["ones4"] = np.ones((P, NCH), np.float32)
    consts["ident"] = np.eye(P, dtype=np.float32)
    m = np.arange(P)
    consts["rep16"] = (m[None, :] % 16 == np.arange(16)[:, None]).astype(np.float32)
    m16 = np.arange(16)
    rep = (m[None, :] % 16 == m16[:, None]).astype(np.float32)
    consts["rep16x2"] = 2.0 * rep
    consts["nrep16"] = -rep
    return consts


def build_nc():
    nc = bacc.Bacc(
        "TRN2",
        target_bir_lowering=False,
        debug=False,
        enable_asserts=False,
        num_swdge_queues=NQ,
    )

    enc_dr = nc.dram_tensor("enc", [BPC, S, H], F32, kind="ExternalInput")
    pit_dr = nc.dram_tensor("pitch", [BPC, S], F32, kind="ExternalInput")
    ene_dr = nc.dram_tensor("energy", [BPC, S], F32, kind="ExternalInput")
    dur_dr = nc.dram_tensor("durt", [BPC, S], F32, kind="ExternalInput")
    tab_dr = {
        nm: nc.dram_tensor(nm, [NBINS, H], BF16, kind="ExternalInput")
        for nm in ("dpt", "det")
    }
    cdr = {
        name: nc.dram_tensor(name, list(arr_shape), F32, kind="ExternalInput")
        for name, arr_shape in (
            ("base", (1, H)),
            ("bndp", (P, 2)),
            ("bnde", (P, 2)),
            ("lt128", (P, P)),
            ("slt128", (P, P)),
            ("slt4", (NCH, NCH)),
            ("lt16", (16, 16)),
            ("ones1", (1, P)),
            ("ones1_16", (1, 16)),
            ("onescol16", (16, 1)),
            ("onecol", (P, 1)),
            ("ones4", (P, NCH)),
            ("ident", (P, P)),
            ("rep16", (16, P)),
            ("rep16x2", (16, P)),
            ("nrep16", (16, P)),
        )
    }
    out_dr = [
        nc.dram_tensor(f"out{b}", [T, H], F32, kind="ExternalOutput")
        for b in range(BPC)
    ]
    # unit u holds y[u//2]; units 1024/1025 are zeros (ragged tail)
    yc_dr = nc.dram_tensor("yc_scr", [BPC * PTU, H], BF16)
    delta_dr = [nc.dram_tensor(f"delta{b}", [DELTA_N, 1], F32) for b in range(BPC)]

    with tile.TileContext(nc) as tc:
        with (
            tc.tile_pool(name="const", bufs=1) as cp,
            tc.tile_pool(name="work", bufs=4) as wp,
            tc.tile_pool(name="gat", bufs=4) as gp,
            tc.tile_pool(name="gf32", bufs=4) as op,
            tc.tile_pool(name="idxp", bufs=BPC) as ip,
            tc.tile_pool(name="pbig", bufs=1, space="PSUM") as pb,
            tc.tile_pool(name="peps", bufs=2, space="PSUM") as pe,
            tc.tile_pool(name="psmall", bufs=2, space="PSUM") as psm,
            tc.tile_pool(name="pmicro", bufs=1, space="PSUM") as pmi,
            tc.tile_pool(name="prep", bufs=2, space="PSUM") as prp,
        ):
            # ---- constants ----
            csb = {}
            for name, dr in cdr.items():
                t_ = cp.tile(list(dr.shape), F32, tag=name)
                nc.sync.dma_start(out=t_[:], in_=dr[:])
                csb[name] = t_
            for nm, dr in tab_dr.items():
                t_ = cp.tile([P, 2, H], BF16, tag=nm)
                nc.sync.dma_start(
                    out=t_[:], in_=dr[:].rearrange("(h p) f -> p h f", p=P)
                )
                csb[nm] = t_
            zt = cp.tile([8, H], BF16)
            nc.gpsimd.memset(zt[:], 0.0)
            zrow = cp.tile([1, DELTA_N], F32)
            nc.gpsimd.memset(zrow[:], 0.0)

            idx_tiles = {}

            def phase0(b):
                # ---- dur load + frame-delta chain (tiny, unblocks gathers) ----
                dur_raw = wp.tile([P, NCH], F32, tag="draw")
                nc.sync.dma_start(
                    out=dur_raw[:], in_=dur_dr[b].rearrange("(c p) -> p c", p=P)
                )

                # ---- dur = max(round_half_even(durt), 1) ----
                MAGIC = float(1 << 23)
                dr0 = wp.tile([P, NCH], F32, tag="dr0")
                nc.vector.tensor_scalar(out=dr0[:], in0=dur_raw[:], scalar1=MAGIC, scalar2=MAGIC, op0=A.add, op1=A.subtract)
                dur_sb = wp.tile([P, NCH], F32, tag="dur")
                nc.vector.tensor_scalar(out=dur_sb[:], in0=dr0[:], scalar1=1.0, scalar2=None, op0=A.max)

                # ---- inclusive cum over tokens (wrapped j = pc*128 + p) ----
                i1_ps = psm.tile([P, NCH], F32, tag="small")
                nc.tensor.matmul(out=i1_ps[:], lhsT=csb["lt128"][:], rhs=dur_sb[:], start=True, stop=True)
                i1_sb = wp.tile([P, NCH], F32, tag="i1")
                nc.vector.tensor_copy(out=i1_sb[:], in_=i1_ps[:])
                tot_ps = psm.tile([NCH, P], F32, tag="small")
                nc.tensor.transpose(out=tot_ps[:], in_=i1_sb[:], identity=csb["ident"][:])
                tot_sb = wp.tile([NCH, P], F32, tag="tot")
                nc.vector.tensor_copy(out=tot_sb[:], in_=tot_ps[:])
                totb_sb = wp.tile([NCH, P], F32, tag="totb")
                nc.vector.tensor_copy(out=totb_sb[:], in_=tot_sb[:, P - 1 : P].to_broadcast([NCH, P]))
                cum_ps = psm.tile([P, NCH], F32, tag="small")
                nc.tensor.matmul(out=cum_ps[:], lhsT=csb["lt128"][:], rhs=dur_sb[:], start=True, stop=False)
                nc.tensor.matmul(out=cum_ps[:], lhsT=totb_sb[:], rhs=csb["slt4"][:], start=False, stop=True)
                cum_i32 = wp.tile([P, NCH], I32, tag="cumi")
                nc.vector.tensor_copy(out=cum_i32[:], in_=cum_ps[:])


                # ---- delta: zero then scatter 1.0 at cum positions ----
                nc.sync.dma_start(
                    out=delta_dr[b][:].rearrange("v o -> o v"), in_=zrow[:]
                )
                for c in range(NCH):
                    nc.gpsimd.indirect_dma_start(
                        out=delta_dr[b][:],
                        out_offset=IndirectOffsetOnAxis(ap=cum_i32[:, c : c + 1], axis=0),
                        in_=csb["onecol"][:],
                        in_offset=None,
                    )

            def phase1(b):
                # ---- heavy loads + variance-adder compute ----
                enc_sb = wp.tile([P, NCH, H], F32, tag="enc")
                nc.sync.dma_start(
                    out=enc_sb[:], in_=enc_dr[b].rearrange("(c p) f -> p c f", p=P)
                )
                vp_row = wp.tile([1, S], F32, tag="vp")
                ve_row = wp.tile([1, S], F32, tag="ve")
                nc.sync.dma_start(out=vp_row[:], in_=pit_dr[b][None, :])
                nc.sync.dma_start(out=ve_row[:], in_=ene_dr[b][None, :])

                # ---- C matrices: C[bin_p, tok] = (boundary[bin] < v[tok]) ----
                vp_ps = pb.tile([P, S], F32, tag="vps")
                nc.tensor.matmul(out=vp_ps[:], lhsT=csb["ones1"][:], rhs=vp_row[:], start=True, stop=True)
                cp0 = wp.tile([P, S], BF16, tag="cp0")
                cp1 = wp.tile([P, S], BF16, tag="cp1")
                nc.vector.tensor_scalar(out=cp0[:], in0=vp_ps[:], scalar1=csb["bndp"][:, 0:1], scalar2=None, op0=A.is_gt)
                nc.vector.tensor_scalar(out=cp1[:], in0=vp_ps[:], scalar1=csb["bndp"][:, 1:2], scalar2=None, op0=A.is_gt)
                ve_ps = pb.tile([P, S], F32, tag="vps")
                nc.tensor.matmul(out=ve_ps[:], lhsT=csb["ones1"][:], rhs=ve_row[:], start=True, stop=True)
                ce0 = wp.tile([P, S], BF16, tag="ce0")
                ce1 = wp.tile([P, S], BF16, tag="ce1")
                nc.vector.tensor_scalar(out=ce0[:], in0=ve_ps[:], scalar1=csb["bnde"][:, 0:1], scalar2=None, op0=A.is_gt)
                nc.vector.tensor_scalar(out=ce1[:], in0=ve_ps[:], scalar1=csb["bnde"][:, 1:2], scalar2=None, op0=A.is_gt)

                # ---- y = enc + ptab[pbin] + etab[ebin] (bf16 scratch rows) ----
                y_sb = wp.tile([P, NCH, H], BF16, tag="y")
                for c in range(NCH):
                    eps = pe.tile([P, H], F32, tag="eps")
                    nc.tensor.matmul(out=eps[:], lhsT=cp0[:, ts(c, P)], rhs=csb["dpt"][:, 0, :], start=True, stop=False)
                    nc.tensor.matmul(out=eps[:], lhsT=cp1[:, ts(c, P)], rhs=csb["dpt"][:, 1, :], start=False, stop=False)
                    nc.tensor.matmul(out=eps[:], lhsT=ce0[:, ts(c, P)], rhs=csb["det"][:, 0, :], start=False, stop=False)
                    nc.tensor.matmul(out=eps[:], lhsT=ce1[:, ts(c, P)], rhs=csb["det"][:, 1, :], start=False, stop=False)
                    nc.tensor.matmul(out=eps[:], lhsT=csb["ones1"][:], rhs=csb["base"][:], start=False, stop=True)
                    nc.vector.tensor_tensor(out=y_sb[:, c, :], in0=eps[:], in1=enc_sb[:, c, :], op=A.add)

                # ---- y rows duplicated into the unit table (+ zero tail) ----
                yc = yc_dr[b * PTU : (b + 1) * PTU, :]
                nc.sync.dma_start(
                    out=yc[0 : 2 * S : 2, :].rearrange("(c p) f -> p c f", p=P),
                    in_=y_sb[:],
                )
                nc.sync.dma_start(
                    out=yc[1 : 2 * S : 2, :].rearrange("(c p) f -> p c f", p=P),
                    in_=y_sb[:],
                )
                nc.sync.dma_start(out=yc[2 * S : 2 * S + 4, :], in_=zt[0:4, :])

            def phase2(b):
                # ---- pair-space frame indexing: position k = s*16+p16 is the
                # frame pair (2k, 2k+1); dpair[p16, s, r] = delta[2k+r] ----
                NPC = NPAIR // 16  # pair columns (128)
                dpair = wp.tile([16, NPC, 2], F32, tag="d16")
                nc.sync.dma_start(
                    out=dpair[:],
                    in_=delta_dr[b][0:T, :].rearrange("(s p r) o -> p s (r o)", p=16, r=2),
                )
                ds = wp.tile([16, NPC], F32, tag="ds")
                nc.vector.tensor_tensor(out=ds[:], in0=dpair[:, :, 0], in1=dpair[:, :, 1], op=A.add)
                # within-column (16 consecutive pairs) inclusive prefix of ds
                fi_ps = prp.tile([16, NPC], F32, tag="rep")
                nc.tensor.matmul(out=fi_ps[:], lhsT=csb["lt16"][:], rhs=ds[:], start=True, stop=True)
                fi_sb = wp.tile([16, NPC], F32, tag="fis")
                nc.vector.tensor_copy(out=fi_sb[:], in_=fi_ps[:])
                # column sums and their exclusive prefix (128 cols)
                cs_ps = pmi.tile([P, 1], F32, tag="micro")
                nc.tensor.matmul(out=cs_ps[:], lhsT=ds[:], rhs=csb["onescol16"][:], start=True, stop=True)
                cs_sb = wp.tile([P, 1], F32, tag="cs")
                nc.vector.tensor_copy(out=cs_sb[:], in_=cs_ps[:])
                ep_ps = pmi.tile([P, 1], F32, tag="micro")
                nc.tensor.matmul(out=ep_ps[:], lhsT=csb["slt128"][:], rhs=cs_sb[:], start=True, stop=True)
                ep_sb = wp.tile([P, 1], F32, tag="ep")
                nc.vector.tensor_copy(out=ep_sb[:], in_=ep_ps[:])
                tr_ps = pmi.tile([1, P], F32, tag="micro")
                nc.tensor.transpose(out=tr_ps[:], in_=ep_sb[:], identity=csb["ident"][:])
                cpfx_row = wp.tile([1, NPC], F32, tag="cpfx")
                # x2: table units are half-rows of the duplicated table
                nc.vector.tensor_scalar(out=cpfx_row[:], in0=tr_ps[:], scalar1=2.0, scalar2=None, op0=A.mult)

                # ---- table unit index: 2*P(k) - delta[2k+1] ----
                rep_ps = prp.tile([P, NPC], F32, tag="rep")
                nc.tensor.matmul(out=rep_ps[:], lhsT=csb["rep16x2"][:], rhs=fi_sb[:], start=True, stop=False)
                nc.tensor.matmul(out=rep_ps[:], lhsT=csb["ones1"][:], rhs=cpfx_row[:], start=False, stop=False)
                nc.tensor.matmul(out=rep_ps[:], lhsT=csb["nrep16"][:], rhs=dpair[:, :, 1], start=False, stop=True)
                idx16 = ip.tile([P, NPC], I16, tag=f"idx{b}")
                nc.vector.tensor_copy(out=idx16[:], in_=rep_ps[:])
                idx_tiles[b] = idx16

            def phase3(b, g, q):
                # ---- gather bf16 frame-pairs from the pair table ----
                idx16 = idx_tiles[b]
                # overlapping windows: unit u reads 2H elems starting at u*H
                ybase = yc_dr[b * PTU : (b + 1) * PTU, :]
                ysrc = AP(
                    tensor=ybase.tensor,
                    offset=ybase.offset,
                    ap=[[H, PTU - 2], [1, 2 * H]],
                )
                g_sb = gp.tile([P, GCHUNK // P, 2 * H], BF16, tag="g")
                dma_sem = nc.alloc_semaphore(f"gat_{b}_{g}")
                nc.gpsimd.dma_gather(
                    out_ap=g_sb[:],
                    in_ap=ysrc,
                    idxs_ap=idx16[:, g * (GCHUNK // 16) : (g + 1) * (GCHUNK // 16)],
                    num_idxs=GCHUNK,
                    num_idxs_reg=GCHUNK,
                    elem_size=2 * H,
                    elem_step=H,
                    prepare_only=True,
                    sem=dma_sem,
                    queue_num=q,
                )
                nc.gpsimd.trigger_dma(count=None, queue_num=q)
                f_sb = op.tile([P, GCHUNK // P, 2 * H], F32, tag="f")
                nc.scalar.wait_ge(dma_sem, 16)
                nc.scalar.activation(out=f_sb[:], in_=g_sb[:], func=ACT_COPY)
                nc.scalar.dma_start(
                    out=out_dr[b][g * 2 * GCHUNK : (g + 1) * 2 * GCHUNK, :].rearrange(
                        "(s p r) f -> p s (r f)", p=P, r=2
                    ),
                    in_=f_sb[:],
                )

            phase0(0)
            q = 0
            for b in range(BPC):
                if b + 1 < BPC:
                    phase0(b + 1)
                phase1(b)
                phase2(b)
                for g in range(NGATHER):
                    phase3(b, g, q % NQ)
                    q += 1

    nc.compile()
    return nc


_NC_CACHE = {}


def _get_nc():
    if "nc" not in _NC_CACHE:
        _NC_CACHE["nc"] = build_nc()
    return _NC_CACHE["nc"]


def make_in_maps(inputs):
    enc = np.ascontiguousarray(np.asarray(inputs["encoder_output"], np.float32))
    pit = np.ascontiguousarray(np.asarray(inputs["pitch_target"], np.float32))
    ene = np.ascontiguousarray(np.asarray(inputs["energy_target"], np.float32))
    dur = np.ascontiguousarray(np.asarray(inputs["duration_target"], np.float32))
    ptab = np.asarray(inputs["pitch_table"], np.float32)
    etab = np.asarray(inputs["energy_table"], np.float32)
    consts = _host_constants(ptab, etab)
    in_maps = []
    for c in range(NCORES):
        sl = slice(c * BPC, (c + 1) * BPC)
        m = dict(consts)
        m["enc"] = enc[sl]
        m["pitch"] = pit[sl]
        m["energy"] = ene[sl]
        m["durt"] = dur[sl]
        in_maps.append(m)
    return in_maps


def run(inputs, trace=False):
    nc = _get_nc()
    in_maps = make_in_maps(inputs)
    res = run_bass_kernel_spmd(nc, in_maps, list(range(NCORES)), trace=trace)
    out = np.empty((B, T, H), np.float32)
    for c in range(NCORES):
        for b in range(BPC):
            out[c * BPC + b] = res.results[c][f"out{b}"]
    return out, res


def kernel(**inputs):
    out, _ = run(inputs, trace=False)
    return out
